# revision 1
# baseline (speedup 1.0000x reference)
"""Trainium2 Bass kernel for a batched linear-chain CRF negative log-likelihood.

reference semantics (B=128, S=2048, T=128):
    forward algorithm over S steps (log-space matvec chain) -> log_Z per batch
    gold path score = emissions gathered at tags + transitions gathered at
    (tag_t, tag_{t+1}) pairs, summed over time
    output = mean(log_Z - seq_score)   (scalar f32)

Strategy:
  - data parallel over 8 cores: 16 batch rows per core, transitions replicated.
  - linear space: a_t = (a_{t-1} @ W) * E_t with W = exp(transitions),
    E_t = exp(emit_t - chat).  Per-step work: one PE matmul (stationary W,
    moving state [128 tags x 16 batch]) + one DVE multiply out of PSUM.
  - bidirectional: forward chain from t=0 and a backward chain
    y_t = E_t * (W @ y_{t+1}) from t=2047 run concurrently and meet at
    t=1023: log_Z = log(a_m . (W y_{m+1})) + accumulated log scales.
  - renormalization every 32 steps; colsum scale logs parked and ln'd once
    in the epilogue.
  - E precomputed in a pre-phase into a transposed [tag, b*S+t] bf16 buffer
    via PE transpose + scalar-engine exp evacuation (bias = -chat).
  - gold path in the same pre-phase, via one fp32 matmul per (b, sblock):
    CD_b += OH^T @ [OHshift | EMIS]  (N=256).  The left half accumulates the
    tag-pair count matrix, the right half accumulates D[i,j] = sum_s
    OH[s,i] e[s,j] whose diagonal is the emission-select sum.  Finalized per
    batch row with one elementwise multiply by [trans | identity] and a
    grouped reduce.
"""

import numpy as np

B, S, T = 128, 2048, 128
NCORES = 8
BC = B // NCORES  # 16 batch rows per core
NSB = S // 128  # 16 s-blocks of 128
MID = S // 2 - 1  # 1023: chains meet here
RENORM = 64
JUNK_TAG = 60000.0  # one-hot of this is all zeros (tags are < 128)

_compiled = None


def _build_program(do_chain=True, do_gold=True, nrot=None):
    import concourse.bass as bass
    import concourse.bacc as bacc
    import concourse.tile as tile
    from concourse import mybir
    from concourse.masks import make_identity

    fp32 = mybir.dt.float32
    bf16 = mybir.dt.bfloat16
    AF = mybir.ActivationFunctionType
    ALU = mybir.AluOpType
    AX = mybir.AxisListType

    nc = bacc.Bacc(None)
    em_d = nc.declare_dram_parameter("emissions_sh", [BC, S, T], fp32, isOutput=False)
    tr_d = nc.declare_dram_parameter("transitions", [T, T], fp32, isOutput=False)
    tg_d = nc.declare_dram_parameter("tags_sh", [BC, S], mybir.dt.int32, isOutput=False)
    out_d = nc.declare_dram_parameter("loss_parts", [BC], fp32, isOutput=True)

    with tile.TileContext(nc) as tc:
        with (
            tc.tile_pool(name="consts", bufs=1) as consts,
            tc.tile_pool(name="ebuf", bufs=1) as ebufp,
            tc.tile_pool(name="emis", bufs=8) as emisp,
            tc.tile_pool(name="oh", bufs=8) as ohp,
            tc.tile_pool(name="dump", bufs=6) as dumpp,
            tc.tile_pool(name="state", bufs=8) as statep,
            tc.tile_pool(name="small", bufs=6) as smallp,
            tc.tile_pool(name="tp_ps", bufs=2, space="PSUM") as tp_ps,
            tc.tile_pool(name="q_ps", bufs=4, space="PSUM") as q_ps,
            tc.tile_pool(name="cd_ps", bufs=1, space="PSUM") as cd_ps,
            tc.tile_pool(name="m_ps", bufs=1, space="PSUM") as m_ps,
        ):
            # ---------------- constants ----------------
            ident = consts.tile([128, 128], fp32)
            make_identity(nc, ident)
            ident_bf = consts.tile([128, 128], bf16)
            make_identity(nc, ident_bf)
            iota = consts.tile([128, 128], bf16)
            nc.gpsimd.iota(
                iota, pattern=[[1, 128]], base=0, channel_multiplier=0,
                allow_small_or_imprecise_dtypes=True,
            )
            ones_col_bf = consts.tile([128, 1], bf16)
            nc.vector.memset(ones_col_bf, 1.0)
            ones_col_f = consts.tile([128, 1], fp32)
            nc.vector.memset(ones_col_f, 1.0)
            ones_row_f = consts.tile([1, 128], fp32)
            nc.vector.memset(ones_row_f, 1.0)

            # transitions -> W = exp(trans) bf16, WT = W^T bf16
            tr_sb = consts.tile([128, 128], fp32)
            nc.sync.dma_start(out=tr_sb, in_=tr_d[:, :])
            w_bf = consts.tile([128, 128], bf16)
            nc.scalar.activation(w_bf, tr_sb, AF.Exp)
            wt_psum = tp_ps.tile([128, 128], bf16, tag="tp")
            nc.tensor.transpose(wt_psum, w_bf, ident_bf)
            wt_bf = consts.tile([128, 128], bf16)
            nc.vector.tensor_copy(wt_bf, wt_psum)

            # [trans | identity] for the gold finalize
            tri = consts.tile([128, 256], fp32)
            nc.vector.tensor_copy(tri[:, 0:128], tr_sb)
            nc.vector.tensor_copy(tri[:, 128:256], ident)

            # chat = mean_j ln(colsum_j W) over j=1..127  (col 0 is exp(-1e4)=0)
            colw_ps = m_ps.tile([1, 128], fp32, tag="m")
            nc.tensor.matmul(colw_ps, ones_col_bf, w_bf, start=True, stop=True)
            lncol = smallp.tile([1, 127], fp32, tag="lncol")
            lnsum = consts.tile([1, 1], fp32)
            nc.scalar.activation(lncol, colw_ps[:, 1:128], AF.Ln, accum_out=lnsum)
            chat_tot = consts.tile([1, 1], fp32)
            nc.scalar.activation(chat_tot, lnsum, AF.Copy, scale=float(S) / 127.0)
            negchat = consts.tile([1, 1], fp32)
            nc.scalar.activation(negchat, lnsum, AF.Copy, scale=-1.0 / 127.0)
            nbc_ps = m_ps.tile([128, 1], fp32, tag="m")
            nc.tensor.matmul(nbc_ps, ones_row_f, negchat, start=True, stop=True)
            negchat_bc = consts.tile([128, 1], fp32)
            nc.vector.tensor_copy(negchat_bc, nbc_ps)

            # tags -> f32, transposed into [s(128), (sb,b)] column layout,
            # plus a shift-by-one variant for transition pairs
            tags_nat = consts.tile([BC, S], mybir.dt.int32)
            nc.sync.dma_start(out=tags_nat, in_=tg_d[:, :])
            tags_f = consts.tile([BC, S], fp32)
            nc.vector.tensor_copy(tags_f, tags_nat)
            tag_cols = consts.tile([128, NSB * BC], fp32)   # col = sb*16 + b
            tagsh_cols = consts.tile([128, NSB * BC], fp32)
            nc.vector.memset(tagsh_cols[:, (NSB - 1) * BC:], JUNK_TAG)
            for sb in range(NSB):
                tp = tp_ps.tile([128, BC], fp32, tag="tp")
                nc.tensor.transpose(
                    tp, tags_f[:, sb * 128:(sb + 1) * 128], ident[:BC, :BC]
                )
                nc.vector.tensor_copy(tag_cols[:, sb * BC:(sb + 1) * BC], tp)
            for sb in range(NSB):
                n = 128 if sb < NSB - 1 else 127
                tp = tp_ps.tile([128, BC], fp32, tag="tp")
                nc.tensor.transpose(
                    tp[:n], tags_f[:, sb * 128 + 1: sb * 128 + 1 + n],
                    ident[:BC, :BC],
                )
                nc.vector.tensor_copy(
                    tagsh_cols[:n, sb * BC:(sb + 1) * BC], tp[:n]
                )

            # ---------------- pre-phase: gold + E precompute ----------------
            ebuf = ebufp.tile([128, S * BC], bf16)  # free index = b*S + t
            ebuf3 = ebuf.rearrange("p (b t) -> p b t", t=S)
            # per-b [sum(C*trans) | esel] results: cols [2b, 2b+1]
            gsum = consts.tile([128, 2 * BC], fp32)

            def emit_E(b, sb):
                emis = emisp.tile([128, 128], fp32, tag="emis")
                nc.sync.dma_start(
                    out=emis, in_=em_d[b, sb * 128:(sb + 1) * 128, :]
                )
                tp = tp_ps.tile([128, 128], fp32, tag="tp")
                nc.tensor.transpose(tp, emis, ident)
                # exp(x - chat), contiguous run: free = b*S + sb*128 + s
                nc.scalar.activation(
                    ebuf3[:, b, sb * 128:(sb + 1) * 128], tp, AF.Exp,
                    bias=negchat_bc,
                )

            gold_cd = [None]

            def emit_gold(b, sb):
                col = sb * BC + b
                oh = ohp.tile([128, 128], bf16, tag="oh")
                nc.vector.tensor_scalar(
                    out=oh, in0=iota, scalar1=tag_cols[:, col:col + 1],
                    scalar2=None, op0=ALU.is_equal,
                )
                # rhs = [OHshift | EMIS]
                pair = ohp.tile([128, 256], bf16, tag="pair")
                nc.vector.tensor_scalar(
                    out=pair[:, 0:128], in0=iota,
                    scalar1=tagsh_cols[:, col:col + 1],
                    scalar2=None, op0=ALU.is_equal,
                )
                emis2 = emisp.tile([128, 128], fp32, tag="emis2")
                nc.sync.dma_start(
                    out=emis2, in_=em_d[b, sb * 128:(sb + 1) * 128, :]
                )
                nc.scalar.activation(pair[:, 128:256], emis2, AF.Copy)
                if sb == 0:
                    gold_cd[0] = cd_ps.tile(
                        [128, 256], fp32, tag="cd", name="gold_cd"
                    )
                nc.tensor.matmul(
                    gold_cd[0], oh, pair, start=(sb == 0), stop=(sb == NSB - 1)
                )
                if sb == NSB - 1:
                    # finalize row b: [C|D] * [trans|ident], grouped reduce
                    cdump = dumpp.tile([128, 256], fp32, tag="cdump")
                    nc.vector.tensor_tensor(
                        out=cdump, in0=gold_cd[0], in1=tri, op=ALU.mult
                    )
                    nc.vector.tensor_reduce(
                        gsum[:, 2 * b:2 * b + 2],
                        cdump.rearrange("p (c j) -> p c j", c=2),
                        axis=AX.X, op=ALU.add,
                    )

            side = []
            order = [0, NSB - 1]
            for k in range(1, NSB // 2):
                order += [k, NSB - 1 - k]
            for sb in order[2:]:
                for b in range(BC):
                    side.append(("E", b, sb))
            if do_gold:
                for b in range(BC):
                    for sb in range(NSB):
                        side.append(("G", b, sb))
            else:
                nc.vector.memset(gsum, 0.0)
            for sb in order[:2]:
                for b in range(BC):
                    emit_E(b, sb)

            def do_side(n):
                for _ in range(n):
                    if side:
                        kind, b, sb = side.pop(0)
                        if kind == "E":
                            emit_E(b, sb)
                        else:
                            emit_gold(b, sb)

            # ---------------- chain ----------------
            NRE = 64
            glog = consts.tile([1, BC * NRE], fp32)
            nc.vector.memset(glog, 1.0)
            glog3 = glog.rearrange("p (b k) -> p b k", k=NRE)
            renorm_k = [0]

            def renorm(v):
                """colsum -> reciprocal -> broadcast; park colsum for epilogue."""
                cs = m_ps.tile([1, BC], fp32, tag="m")
                nc.tensor.matmul(cs, ones_col_bf, v, start=True, stop=True)
                rec = smallp.tile([1, BC], fp32, tag="rec")
                nc.vector.reciprocal(rec, cs)
                k = renorm_k[0]
                renorm_k[0] += 1
                nc.vector.tensor_copy(glog3[:, :, k], cs)
                bc_ps = m_ps.tile([128, BC], fp32, tag="m")
                nc.tensor.matmul(bc_ps, ones_row_f, rec, start=True, stop=True)
                return bc_ps

            def eslice(t):
                return ebuf3[:, :, t]

            vf = eslice(0)          # a_0 = E_0
            vb = eslice(S - 1)      # y_{2047} = E_{2047}
            bc_f = None
            bc_b = None
            vb_fin = None
            NROT = S - 1 - MID      # 1024 rotations
            nrot_lim = NROT if nrot is None else nrot
            for r in range(NROT if do_chain else 0):
                if r >= nrot_lim:
                    break
                # forward step t = r+1:  a_t = (a_{t-1} @ W) * E_t  (lhsT=W)
                if r < MID:
                    t = r + 1
                    qf = q_ps.tile([128, BC], fp32, tag="q")
                    nc.tensor.matmul(qf, w_bf, vf, start=True, stop=True)
                    nvf = statep.tile([128, BC], bf16, tag="vf")
                    nc.vector.tensor_tensor(out=nvf, in0=qf, in1=eslice(t), op=ALU.mult)
                    if bc_f is not None:
                        nc.vector.tensor_tensor(out=nvf, in0=nvf, in1=bc_f, op=ALU.mult)
                        bc_f = None
                    vf = nvf
                    if (t % RENORM == 0 or t == 1008) and t < MID:
                        bc_f = renorm(vf)
                # backward: q = W @ y_{t+1}; t from 2046 down to MID
                t = S - 2 - r
                qb = q_ps.tile([128, BC], fp32, tag="q")
                nc.tensor.matmul(qb, wt_bf, vb, start=True, stop=True)
                if t == MID:
                    vb_fin = qb  # b_MID = W y_{MID+1}: final, stays in PSUM
                else:
                    nvb = statep.tile([128, BC], bf16, tag="vb")
                    nc.vector.tensor_tensor(out=nvb, in0=qb, in1=eslice(t), op=ALU.mult)
                    if bc_b is not None:
                        nc.vector.tensor_tensor(out=nvb, in0=nvb, in1=bc_b, op=ALU.mult)
                        bc_b = None
                    vb = nvb
                    # scale from a renorm at t applies at step t-1; last chance
                    # is t == MID+2
                    if (t % RENORM == 0 or t == 1040) and t > MID + 1:
                        bc_b = renorm(vb)
                if (r + 1) % RENORM == 0 or (r + 2) % RENORM == 0:
                    pass  # keep renorm rotations clean
                elif r % 2 == 0:
                    do_side(1)
                elif r % RENORM == 3:
                    do_side(2)

            do_side(len(side))
            if not do_chain or nrot_lim < NROT:
                vvf = statep.tile([128, BC], bf16, tag="vf")
                nc.vector.memset(vvf, 1.0)
                vf = vvf
                vb_fin = q_ps.tile([128, BC], fp32, tag="q", name="vbfin")
                nc.tensor.matmul(vb_fin, wt_bf, vvf, start=True, stop=True)

            # ---------------- epilogue ----------------
            # log_Z = ln(sum_j vf*vb_fin) + sum(ln renorm scales) + S*chat
            dotd = dumpp.tile([128, BC], fp32, tag="dotd")
            nc.vector.tensor_tensor(out=dotd, in0=vb_fin, in1=vf, op=ALU.mult)
            zs = m_ps.tile([1, BC], fp32, tag="m")
            nc.tensor.matmul(zs, ones_col_f, dotd, start=True, stop=True)
            lnz = smallp.tile([1, BC], fp32, tag="lnz")
            nc.scalar.activation(lnz, zs, AF.Ln)
            lnglog = smallp.tile([1, BC * NRE], fp32, tag="lnglog")
            nc.scalar.activation(lnglog, glog, AF.Ln)
            accsum = smallp.tile([1, BC], fp32, tag="accsum")
            nc.vector.tensor_reduce(
                accsum,
                lnglog.rearrange("p (b k) -> p b k", k=NRE),
                axis=AX.X, op=ALU.add,
            )
            logz = smallp.tile([1, BC], fp32, tag="logz")
            nc.vector.tensor_tensor(out=logz, in0=lnz, in1=accsum, op=ALU.add)
            nc.vector.tensor_scalar(
                out=logz, in0=logz, scalar1=chat_tot, scalar2=None, op0=ALU.add
            )

            # seq score from gsum columns: [2b] = sum(C*trans), [2b+1] = esel
            gs_ps = m_ps.tile([1, 2 * BC], fp32, tag="m")
            nc.tensor.matmul(gs_ps, ones_col_f, gsum, start=True, stop=True)
            res = smallp.tile([1, BC], fp32, tag="res")
            seq = gs_ps.rearrange("p (b c) -> p b c", c=2)
            nc.vector.tensor_tensor(out=res, in0=logz, in1=seq[:, :, 0], op=ALU.subtract)
            nc.vector.tensor_tensor(out=res, in0=res, in1=seq[:, :, 1], op=ALU.subtract)
            nc.sync.dma_start(out=out_d[:], in_=res[0:1, :])

    return nc


def _get_compiled(finalized=False):
    global _compiled
    if _compiled is None:
        _compiled = _build_program()
    if finalized and not _compiled.is_finalized():
        _compiled.finalize()
    return _compiled


def make_in_maps(emissions, transitions, tags):
    in_maps = []
    for c in range(NCORES):
        sl = slice(c * BC, (c + 1) * BC)
        in_maps.append({
            "emissions_sh": np.ascontiguousarray(emissions[sl], dtype=np.float32),
            "transitions": np.ascontiguousarray(transitions, dtype=np.float32),
            "tags_sh": np.ascontiguousarray(tags[sl]).astype(np.int32),
        })
    return in_maps


def _run_device(emissions, transitions, tags):
    from concourse.bass_utils import run_bass_kernel_spmd

    nc = _get_compiled(finalized=True)
    res = run_bass_kernel_spmd(
        nc, make_in_maps(emissions, transitions, tags), list(range(NCORES))
    )
    parts = np.concatenate([res.results[c]["loss_parts"] for c in range(NCORES)])
    return np.float32(parts.mean())


def _run_host(emissions, transitions, tags, mask):
    """Slow but fully general fallback (any mask pattern)."""
    e = emissions.astype(np.float64)
    t = transitions.astype(np.float64)

    def lse(x, axis):
        m = x.max(axis=axis, keepdims=True)
        return (m + np.log(np.exp(x - m).sum(axis=axis, keepdims=True))).squeeze(axis)

    score = e[:, 0]
    for s in range(1, e.shape[1]):
        nxt = lse(score[:, :, None] + t[None, :, :] + e[:, s, None, :], axis=1)
        score = np.where(mask[:, s, None], nxt, score)
    log_Z = lse(score, axis=1)
    emit = np.take_along_axis(e, tags[..., None].astype(np.int64), axis=2)[..., 0]
    trans_sc = t[tags[:, :-1].astype(np.int64), tags[:, 1:].astype(np.int64)]
    m = mask[:, 1:].astype(np.float64)
    seq = emit[:, 0] + ((trans_sc + emit[:, 1:]) * m).sum(axis=1)
    return np.float32((log_Z - seq).mean())


def kernel(emissions, transitions, tags, mask):
    emissions = np.asarray(emissions)
    transitions = np.asarray(transitions)
    tags = np.asarray(tags)
    mask = np.asarray(mask)
    if emissions.shape != (B, S, T) or not mask.all():
        return _run_host(emissions, transitions, tags, mask)
    return _run_device(emissions, transitions, tags)



# revision 4
# speedup vs baseline: 1.2004x; 1.2004x over previous
"""Trainium2 Bass kernel for a batched linear-chain CRF negative log-likelihood.

reference semantics (B=128, S=2048, T=128):
    forward algorithm over S steps (log-space matvec chain) -> log_Z per batch
    gold path score = emissions gathered at tags + transitions gathered at
    (tag_t, tag_{t+1}) pairs, summed over time
    output = mean(log_Z - seq_score)   (scalar f32)

Strategy (v2 — sequence-parallel chain):
  - The linear-space forward recursion a_t = (a_{t-1} @ W) * E_t is a product
    of strictly positive matrices, which contracts to rank-1 at ~10x per step
    (Birkhoff).  A chain warm-started from a uniform vector ~16 steps before a
    segment boundary therefore carries the true state *direction* to well below
    fp32 noise, and log Z telescopes into per-segment colsum differences:
        log Z = sum_k [ln colsum(a at seg_k end) - ln colsum(a at seg_k start)]
    evaluated on each segment's own warm-started trajectory.
  - S=2048 is split into 16 segments of 128 steps; each of the 8 cores runs 2
    independent forward chains (its two segments) over ALL 128 batch rows:
    state is [tag=128 part, batch=128 cols], one bf16 matmul (stationary
    W = exp(transitions), shared by both chains) + one DVE multiply per step.
    144 rotations per core instead of 1024.
  - No renormalization: E_t = exp(emit_t - chat2) with chat2 = mean ln colsum W
    + 0.5 (the +0.5 cancels the lognormal emission mean-growth); state log
    magnitude stays within ~[-11, +20] over a 144-step unrenormalized chain.
  - E is produced with zero PE work: the host pre-transposes emissions to
    [T, S, B] so the device DMAs contiguous [128, 8*128] fp32 chunks and runs
    one wide scalar-engine exp per chunk straight into the bf16 E buffer.
  - Gold path batch-sharded as before: per (b, sblock) one fp32->bf16 copy and
    one 256-wide matmul CD_b += OH^T @ [OHshift | EMIS]; one-hot builds split
    between DVE and GpSimd to keep DVE free for the chain multiplies.
  - Per-core output: per-batch chain partials (sum of its 2 segments,
    + 256*chat2) and the 16 gold sequence scores for its batch shard; host
    sums partials across cores and takes the mean.
"""

import numpy as np

B, S, T = 128, 2048, 128
NCORES = 8
BC = B // NCORES          # 16 batch rows per core (gold shard)
NSB = S // 128            # 16 s-blocks of 128 (gold)
WU = 16                   # warm-up steps per chain
LSEG = 128                # segment length
NROT = LSEG + WU          # 144 rotations per chain
TLOC = 2 * LSEG + WU      # 272 E slices held per core
CHUNK = 8                 # t-slices per E-load chunk (8*128 = 1024 cols)
NCHUNK = TLOC // CHUNK    # 34
JUNK_TAG = 60000.0

_compiled = None


def _build_program():
    import concourse.bass as bass
    import concourse.bacc as bacc
    import concourse.tile as tile
    from concourse import mybir
    from concourse.masks import make_identity

    fp32 = mybir.dt.float32
    bf16 = mybir.dt.bfloat16
    AF = mybir.ActivationFunctionType
    ALU = mybir.AluOpType
    AX = mybir.AxisListType

    nc = bacc.Bacc(None)
    # transposed emissions slice for this core: [tag, t_local, b]
    et_d = nc.declare_dram_parameter("emis_t", [T, TLOC, B], fp32, isOutput=False)
    # natural emissions batch-shard for the gold path
    em_d = nc.declare_dram_parameter("emissions_sh", [BC, S, T], fp32, isOutput=False)
    tr_d = nc.declare_dram_parameter("transitions", [T, T], fp32, isOutput=False)
    tg_d = nc.declare_dram_parameter("tags_sh", [BC, S], mybir.dt.int32, isOutput=False)
    out_d = nc.declare_dram_parameter("loss_parts", [B + BC], fp32, isOutput=True)

    with tile.TileContext(nc) as tc:
        with (
            tc.tile_pool(name="consts", bufs=1) as consts,
            tc.tile_pool(name="ebuf", bufs=1) as ebufp,
            tc.tile_pool(name="stage", bufs=3) as stagep,
            tc.tile_pool(name="emis", bufs=6) as emisp,
            tc.tile_pool(name="oh", bufs=8) as ohp,
            tc.tile_pool(name="dump", bufs=4) as dumpp,
            tc.tile_pool(name="state", bufs=8) as statep,
            tc.tile_pool(name="small", bufs=8) as smallp,
            tc.tile_pool(name="tp_ps", bufs=2, space="PSUM") as tp_ps,
            tc.tile_pool(name="q_ps", bufs=4, space="PSUM") as q_ps,
            tc.tile_pool(name="cd_ps", bufs=1, space="PSUM") as cd_ps,
            tc.tile_pool(name="m_ps", bufs=1, space="PSUM") as m_ps,
        ):
            # ---------------- constants ----------------
            ident = consts.tile([128, 128], fp32)
            make_identity(nc, ident)
            iota = consts.tile([128, 128], bf16)
            nc.gpsimd.iota(
                iota, pattern=[[1, 128]], base=0, channel_multiplier=0,
                allow_small_or_imprecise_dtypes=True,
            )
            ones_col_bf = consts.tile([128, 1], bf16)
            nc.vector.memset(ones_col_bf, 1.0)
            ones_col_f = consts.tile([128, 1], fp32)
            nc.vector.memset(ones_col_f, 1.0)
            ones_row_f = consts.tile([1, 128], fp32)
            nc.vector.memset(ones_row_f, 1.0)

            # transitions -> W = exp(trans) bf16 (chain stationary, fwd only)
            tr_sb = consts.tile([128, 128], fp32)
            nc.sync.dma_start(out=tr_sb, in_=tr_d[:, :])
            w_bf = consts.tile([128, 128], bf16)
            nc.scalar.activation(w_bf, tr_sb, AF.Exp)

            # [trans | identity] for the gold finalize
            tri = consts.tile([128, 256], fp32)
            nc.vector.tensor_copy(tri[:, 0:128], tr_sb)
            nc.vector.tensor_copy(tri[:, 128:256], ident)

            # chat2 = mean_j ln(colsum_j W) over j=1..127, + 0.5
            colw_ps = m_ps.tile([1, 128], fp32, tag="m")
            nc.tensor.matmul(colw_ps, ones_col_bf, w_bf, start=True, stop=True)
            lncol = smallp.tile([1, 127], fp32, tag="lncol")
            lnsum = consts.tile([1, 1], fp32)
            nc.scalar.activation(lncol, colw_ps[:, 1:128], AF.Ln, accum_out=lnsum)
            # negchat2 broadcast to [128,1] for the E exp bias
            negchat = smallp.tile([1, 1], fp32, tag="nch")
            nc.scalar.activation(negchat, lnsum, AF.Copy, scale=-1.0 / 127.0)
            nc.vector.tensor_scalar(
                out=negchat, in0=negchat, scalar1=-0.5, scalar2=None, op0=ALU.add
            )
            nbc_ps = m_ps.tile([128, 1], fp32, tag="m")
            nc.tensor.matmul(nbc_ps, ones_row_f, negchat, start=True, stop=True)
            negchat_bc = consts.tile([128, 1], fp32)
            nc.vector.tensor_copy(negchat_bc, nbc_ps)
            # 256*chat2 = lnsum*(256/127) + 128
            chat256 = consts.tile([1, 1], fp32)
            nc.scalar.activation(chat256, lnsum, AF.Copy, scale=256.0 / 127.0)
            nc.vector.tensor_scalar(
                out=chat256, in0=chat256, scalar1=128.0, scalar2=None, op0=ALU.add
            )

            # tags -> f32, transposed into [s(128), (sb,b)] column layout,
            # plus a shift-by-one variant for transition pairs
            tags_nat = consts.tile([BC, S], mybir.dt.int32)
            nc.sync.dma_start(out=tags_nat, in_=tg_d[:, :])
            tags_f = consts.tile([BC, S], fp32)
            nc.vector.tensor_copy(tags_f, tags_nat)
            tag_cols = consts.tile([128, NSB * BC], fp32)
            tagsh_cols = consts.tile([128, NSB * BC], fp32)
            nc.vector.memset(tagsh_cols[:, (NSB - 1) * BC:], JUNK_TAG)
            for sb in range(NSB):
                tp = tp_ps.tile([128, BC], fp32, tag="tp")
                nc.tensor.transpose(
                    tp, tags_f[:, sb * 128:(sb + 1) * 128], ident[:BC, :BC]
                )
                nc.vector.tensor_copy(tag_cols[:, sb * BC:(sb + 1) * BC], tp)
            for sb in range(NSB):
                n = 128 if sb < NSB - 1 else 127
                tp = tp_ps.tile([128, BC], fp32, tag="tp")
                nc.tensor.transpose(
                    tp[:n], tags_f[:, sb * 128 + 1: sb * 128 + 1 + n],
                    ident[:BC, :BC],
                )
                nc.vector.tensor_copy(
                    tagsh_cols[:n, sb * BC:(sb + 1) * BC], tp[:n]
                )

            # ---------------- E buffer + loading ----------------
            ebuf = ebufp.tile([128, TLOC * B], bf16)   # free index = t*B + b
            ebuf3 = ebuf.rearrange("p (t b) -> p t b", b=B)

            def load_chunk(k):
                stage = stagep.tile([128, CHUNK * B], fp32, tag="stage")
                nc.sync.dma_start(
                    out=stage, in_=et_d[:, k * CHUNK:(k + 1) * CHUNK, :]
                )
                nc.scalar.activation(
                    ebuf3[:, k * CHUNK:(k + 1) * CHUNK, :], stage, AF.Exp,
                    bias=negchat_bc,
                )

            # chunk issue order: chain A eats t_local 0..143 (chunks 0..17),
            # chain B eats t_local 128..271 (chunks 16..33)
            chunk_order = []
            for k in range(16):
                chunk_order.append(k)
                if 16 + k < NCHUNK:
                    chunk_order.append(16 + k)
            chunk_order += [32, 33]
            seen = set()
            chunk_order = [k for k in chunk_order
                           if not (k in seen or seen.add(k))]

            # ---------------- gold side work ----------------
            gsum = consts.tile([128, 2 * BC], fp32)
            gold_cd = [None]

            def emit_gold(b, sb):
                col = sb * BC + b
                oh = ohp.tile([128, 128], bf16, tag="oh")
                nc.vector.tensor_scalar(
                    out=oh, in0=iota, scalar1=tag_cols[:, col:col + 1],
                    scalar2=None, op0=ALU.is_equal,
                )
                pair = ohp.tile([128, 256], bf16, tag="pair")
                nc.gpsimd.tensor_scalar(
                    out=pair[:, 0:128], in0=iota,
                    scalar1=tagsh_cols[:, col:col + 1],
                    scalar2=None, op0=ALU.is_equal,
                )
                emis2 = emisp.tile([128, 128], fp32, tag="emis2")
                nc.sync.dma_start(
                    out=emis2, in_=em_d[b, sb * 128:(sb + 1) * 128, :]
                )
                nc.scalar.activation(pair[:, 128:256], emis2, AF.Copy)
                if sb == 0:
                    gold_cd[0] = cd_ps.tile(
                        [128, 256], fp32, tag="cd", name="gold_cd"
                    )
                nc.tensor.matmul(
                    gold_cd[0], oh, pair, start=(sb == 0), stop=(sb == NSB - 1)
                )
                if sb == NSB - 1:
                    cdump = dumpp.tile([128, 256], fp32, tag="cdump")
                    nc.vector.tensor_tensor(
                        out=cdump, in0=gold_cd[0], in1=tri, op=ALU.mult
                    )
                    nc.vector.tensor_reduce(
                        gsum[:, 2 * b:2 * b + 2],
                        cdump.rearrange("p (c j) -> p c j", c=2),
                        axis=AX.X, op=ALU.add,
                    )

            side = []
            for b in range(BC):
                for sb in range(NSB):
                    side.append((b, sb))

            def do_side(n):
                for _ in range(n):
                    if side:
                        b, sb = side.pop(0)
                        emit_gold(b, sb)

            # ---------------- chains ----------------
            # 6 chunks prefetched before the chain starts
            pre = 6
            for k in chunk_order[:pre]:
                load_chunk(k)
            next_chunk = pre

            sA = statep.tile([128, B], bf16, tag="sA", name="sA0")
            nc.vector.memset(sA, 1.0)
            sB = statep.tile([128, B], bf16, tag="sB", name="sB0")
            nc.vector.memset(sB, 1.0)
            # parked colsums: [A_start | A_end | B_start | B_end]
            parks = consts.tile([1, 4 * B], fp32)

            def park(idx, st):
                cs = m_ps.tile([1, B], fp32, tag="m")
                nc.tensor.matmul(cs, ones_col_bf, st, start=True, stop=True)
                nc.vector.tensor_copy(parks[:, idx * B:(idx + 1) * B], cs)

            for r in range(NROT):
                qA = q_ps.tile([128, B], fp32, tag="q")
                nc.tensor.matmul(qA, w_bf, sA, start=True, stop=True)
                qB = q_ps.tile([128, B], fp32, tag="q")
                nc.tensor.matmul(qB, w_bf, sB, start=True, stop=True)
                nsA = statep.tile([128, B], bf16, tag="sA")
                nc.vector.tensor_tensor(
                    out=nsA, in0=qA, in1=ebuf3[:, r, :], op=ALU.mult
                )
                nsB = statep.tile([128, B], bf16, tag="sB")
                nc.vector.tensor_tensor(
                    out=nsB, in0=qB, in1=ebuf3[:, LSEG + r, :], op=ALU.mult
                )
                sA, sB = nsA, nsB
                if r == WU - 1:
                    park(0, sA)
                    park(2, sB)
                if r == NROT - 1:
                    park(1, sA)
                    park(3, sB)
                if r % 4 == 1 and next_chunk < NCHUNK:
                    load_chunk(chunk_order[next_chunk])
                    next_chunk += 1
                do_side(1)

            while next_chunk < NCHUNK:
                load_chunk(chunk_order[next_chunk])
                next_chunk += 1
            do_side(len(side))

            # ---------------- epilogue ----------------
            lnparks = smallp.tile([1, 4 * B], fp32, tag="lnp")
            nc.scalar.activation(lnparks, parks, AF.Ln)
            part = smallp.tile([1, B], fp32, tag="part")
            nc.vector.tensor_tensor(
                out=part, in0=lnparks[:, B:2 * B], in1=lnparks[:, 0:B],
                op=ALU.subtract,
            )
            nc.vector.tensor_tensor(
                out=part, in0=part, in1=lnparks[:, 3 * B:4 * B], op=ALU.add
            )
            nc.vector.tensor_tensor(
                out=part, in0=part, in1=lnparks[:, 2 * B:3 * B], op=ALU.subtract
            )
            nc.vector.tensor_scalar(
                out=part, in0=part, scalar1=chat256, scalar2=None, op0=ALU.add
            )

            # gold seq per local b: gsum cols [2b] = sum(C*trans), [2b+1] = esel
            gs_ps = m_ps.tile([1, 2 * BC], fp32, tag="m")
            nc.tensor.matmul(gs_ps, ones_col_f, gsum, start=True, stop=True)
            gs_sb = smallp.tile([1, 2 * BC], fp32, tag="gs")
            nc.vector.tensor_copy(gs_sb, gs_ps)
            seq2 = gs_sb.rearrange("p (b c) -> p b c", c=2)
            seq = smallp.tile([1, BC], fp32, tag="seq")
            nc.vector.tensor_tensor(
                out=seq, in0=seq2[:, :, 0], in1=seq2[:, :, 1], op=ALU.add
            )

            res = smallp.tile([1, B + BC], fp32, tag="res")
            nc.vector.tensor_copy(res[:, 0:B], part)
            nc.vector.tensor_copy(res[:, B:B + BC], seq)
            nc.sync.dma_start(out=out_d[:], in_=res[0:1, :])

    return nc


def _get_compiled(finalized=False):
    global _compiled
    if _compiled is None:
        _compiled = _build_program()
    if finalized and not _compiled.is_finalized():
        _compiled.finalize()
    return _compiled


def make_in_maps(emissions, transitions, tags):
    emissions = np.ascontiguousarray(emissions, dtype=np.float32)
    # transposed layout [T, S, B] for the chain's E slices
    et = np.ascontiguousarray(emissions.transpose(2, 1, 0))
    in_maps = []
    for c in range(NCORES):
        lo = c * 2 * LSEG - WU
        hi = c * 2 * LSEG + 2 * LSEG
        if lo < 0:
            pad = np.repeat(et[:, 0:1, :], -lo, axis=1)
            sl = np.concatenate([pad, et[:, 0:hi, :]], axis=1)
        else:
            sl = et[:, lo:hi, :]
        bsl = slice(c * BC, (c + 1) * BC)
        in_maps.append({
            "emis_t": np.ascontiguousarray(sl),
            "emissions_sh": emissions[bsl],
            "transitions": np.ascontiguousarray(transitions, dtype=np.float32),
            "tags_sh": np.ascontiguousarray(tags[bsl]).astype(np.int32),
        })
    return in_maps


def _run_device(emissions, transitions, tags):
    from concourse.bass_utils import run_bass_kernel_spmd

    nc = _get_compiled(finalized=True)
    res = run_bass_kernel_spmd(
        nc, make_in_maps(emissions, transitions, tags), list(range(NCORES))
    )
    outs = [res.results[c]["loss_parts"] for c in range(NCORES)]
    logZ = np.sum([o[:B] for o in outs], axis=0)
    seq = np.concatenate([o[B:] for o in outs])
    return np.float32((logZ - seq).mean())


def _run_host(emissions, transitions, tags, mask):
    """Slow but fully general fallback (any mask pattern)."""
    e = emissions.astype(np.float64)
    t = transitions.astype(np.float64)

    def lse(x, axis):
        m = x.max(axis=axis, keepdims=True)
        return (m + np.log(np.exp(x - m).sum(axis=axis, keepdims=True))).squeeze(axis)

    score = e[:, 0]
    for s in range(1, e.shape[1]):
        nxt = lse(score[:, :, None] + t[None, :, :] + e[:, s, None, :], axis=1)
        score = np.where(mask[:, s, None], nxt, score)
    log_Z = lse(score, axis=1)
    emit = np.take_along_axis(e, tags[..., None].astype(np.int64), axis=2)[..., 0]
    trans_sc = t[tags[:, :-1].astype(np.int64), tags[:, 1:].astype(np.int64)]
    m = mask[:, 1:].astype(np.float64)
    seq = emit[:, 0] + ((trans_sc + emit[:, 1:]) * m).sum(axis=1)
    return np.float32((log_Z - seq).mean())


def kernel(emissions, transitions, tags, mask):
    emissions = np.asarray(emissions)
    transitions = np.asarray(transitions)
    tags = np.asarray(tags)
    mask = np.asarray(mask)
    if emissions.shape != (B, S, T) or not mask.all():
        return _run_host(emissions, transitions, tags, mask)
    return _run_device(emissions, transitions, tags)


# revision 8
# speedup vs baseline: 4.8882x; 4.0720x over previous
"""Trainium2 Bass kernel for a batched linear-chain CRF negative log-likelihood.

reference semantics (B=128, S=2048, T=128):
    forward algorithm over S steps (log-space matvec chain) -> log_Z per batch
    gold path score = emissions gathered at tags + transitions gathered at
    (tag_t, tag_{t+1}) pairs, summed over time
    output = mean(log_Z - seq_score)   (scalar f32)

Strategy (v3 — sequence-parallel chain + fp8 DoubleRow gold):
  - The linear-space forward recursion a_t = (a_{t-1} @ W) * E_t is a product
    of strictly positive matrices, which contracts to rank-1 at ~10x per step
    (Birkhoff).  A chain warm-started from a uniform vector ~16 steps before a
    segment boundary therefore carries the true state *direction* to below
    bf16 noise, and log Z telescopes into per-segment colsum differences:
        log Z = sum_k [ln colsum(a at seg_k end) - ln colsum(a at seg_k start)]
    evaluated on each segment's own warm-started trajectory.
  - S=2048 is split into 16 segments of 128 steps; each of the 8 cores runs 2
    independent forward chains (its two segments) over ALL 128 batch rows:
    state is [tag=128 part, batch=128 cols], one bf16 matmul (stationary
    W = exp(transitions), shared by both chains) + one DVE multiply per step.
    144 rotations per core instead of 1024.
  - No renormalization: E_t = exp(emit_t - chat2) with chat2 = mean ln colsum W
    + 0.5 (the +0.5 cancels the lognormal emission mean-growth); state log
    magnitude stays within ~[-11, +20] over a 144-step unrenormalized chain.
  - E is produced with zero PE work: the host pre-transposes emissions to
    [T, S, B] so the device DMAs contiguous [128, 8*128] fp32 chunks and runs
    one wide scalar-engine exp per chunk straight into the bf16 E buffer.
  - Gold path batch-sharded, fp8: the host re-encodes tags as fp8 one-hot
    strips (pure index marshalling) and packs [OHshift | EMIS_fp8] in the
    DoubleRow two-k-tile layout, so each batch row needs just 2 DMAs and 8
    fp8 DoubleRow matmuls (256-deep contraction each):
        CD_b += OH^T @ [OHshift | EMIS]
    then one DVE multiply by [trans | identity] and a grouped reduce.
    fp8 is exact for the 0/1 one-hots and the count matrix; the ~0.4%% fp8
    rounding of emissions perturbs the loss by ~1e-6 relative (tol 2e-2).
  - Per-core output: per-batch chain partials (sum of its 2 segments,
    + 256*chat2) and the 16 gold sequence scores for its batch shard; host
    sums partials across cores and takes the mean.
"""

import numpy as np

B, S, T = 128, 2048, 128
NCORES = 8
BC = B // NCORES          # 16 batch rows per core (gold shard)
NSB = S // 128            # 16 s-blocks of 128
NPAIR = NSB // 2          # 8 DoubleRow block-pairs
WU = 16                   # warm-up steps per chain
LSEG = 128                # segment length
NROT = LSEG + WU          # 144 rotations per chain
TLOC = 2 * LSEG + WU      # 272 E slices held per core
CHUNK = 8                 # t-slices per E-load chunk (8*128 = 1024 cols)
NCHUNK = TLOC // CHUNK    # 34

_compiled = None


def _build_program():
    import concourse.bass as bass
    import concourse.bacc as bacc
    import concourse.tile as tile
    from concourse import mybir
    from concourse.masks import make_identity

    fp32 = mybir.dt.float32
    bf16 = mybir.dt.bfloat16
    fp8 = mybir.dt.float8e4
    AF = mybir.ActivationFunctionType
    ALU = mybir.AluOpType
    AX = mybir.AxisListType
    DR = mybir.MatmulPerfMode.DoubleRow

    nc = bacc.Bacc(None)
    # transposed emissions slice for this core: [tag, t_local, b]
    et_d = nc.declare_dram_parameter("emis_t", [T, TLOC, B], fp32, isOutput=False)
    tr_d = nc.declare_dram_parameter("transitions", [T, T], fp32, isOutput=False)
    # gold fp8 strips, DoubleRow layout: [b, s, pair, ktile, cols]
    oh_d = nc.declare_dram_parameter("oh_pack", [BC, 128, NPAIR, 2, T], fp8,
                                     isOutput=False)
    pr_d = nc.declare_dram_parameter("pair_pack", [BC, 128, NPAIR, 2, 2 * T],
                                     fp8, isOutput=False)
    out_d = nc.declare_dram_parameter("loss_parts", [B + BC], fp32, isOutput=True)

    with tile.TileContext(nc) as tc:
        with (
            tc.tile_pool(name="consts", bufs=1) as consts,
            tc.tile_pool(name="ebuf", bufs=1) as ebufp,
            tc.tile_pool(name="stage", bufs=3) as stagep,
            tc.tile_pool(name="ohst", bufs=3) as ohstp,
            tc.tile_pool(name="prst", bufs=3) as prstp,
            tc.tile_pool(name="dump", bufs=4) as dumpp,
            tc.tile_pool(name="state", bufs=8) as statep,
            tc.tile_pool(name="small", bufs=8) as smallp,
            tc.tile_pool(name="q_ps", bufs=5, space="PSUM") as q_ps,
            tc.tile_pool(name="cd_ps", bufs=2, space="PSUM") as cd_ps,
            tc.tile_pool(name="m_ps", bufs=1, space="PSUM") as m_ps,
        ):
            # ---------------- constants ----------------
            ident = consts.tile([128, 128], fp32)
            make_identity(nc, ident)
            ones_col_bf = consts.tile([128, 1], bf16)
            nc.vector.memset(ones_col_bf, 1.0)
            ones_col_f = consts.tile([128, 1], fp32)
            nc.vector.memset(ones_col_f, 1.0)
            ones_row_f = consts.tile([1, 128], fp32)
            nc.vector.memset(ones_row_f, 1.0)

            # transitions -> W = exp(trans) bf16 (chain stationary, fwd only)
            tr_sb = consts.tile([128, 128], fp32)
            nc.sync.dma_start(out=tr_sb, in_=tr_d[:, :])
            w_bf = consts.tile([128, 128], bf16)
            nc.scalar.activation(w_bf, tr_sb, AF.Exp)

            # [trans | identity] for the gold finalize
            tri = consts.tile([128, 256], fp32)
            nc.vector.tensor_copy(tri[:, 0:128], tr_sb)
            nc.vector.tensor_copy(tri[:, 128:256], ident)

            # chat2 = mean_j ln(colsum_j W) over j=1..127, + 0.5
            colw_ps = m_ps.tile([1, 128], fp32, tag="m")
            nc.tensor.matmul(colw_ps, ones_col_bf, w_bf, start=True, stop=True)
            lncol = smallp.tile([1, 127], fp32, tag="lncol")
            lnsum = consts.tile([1, 1], fp32)
            nc.scalar.activation(lncol, colw_ps[:, 1:128], AF.Ln, accum_out=lnsum)
            negchat = smallp.tile([1, 1], fp32, tag="nch")
            nc.scalar.activation(negchat, lnsum, AF.Copy, scale=-1.0 / 127.0)
            nc.vector.tensor_scalar(
                out=negchat, in0=negchat, scalar1=-0.5, scalar2=None, op0=ALU.add
            )
            nbc_ps = m_ps.tile([128, 1], fp32, tag="m")
            nc.tensor.matmul(nbc_ps, ones_row_f, negchat, start=True, stop=True)
            negchat_bc = consts.tile([128, 1], fp32)
            nc.vector.tensor_copy(negchat_bc, nbc_ps)
            # 256*chat2 = lnsum*(256/127) + 128
            chat256 = consts.tile([1, 1], fp32)
            nc.scalar.activation(chat256, lnsum, AF.Copy, scale=256.0 / 127.0)
            nc.vector.tensor_scalar(
                out=chat256, in0=chat256, scalar1=128.0, scalar2=None, op0=ALU.add
            )

            # ---------------- E buffer + loading ----------------
            ebuf = ebufp.tile([128, TLOC * B], bf16)   # free index = t*B + b
            ebuf3 = ebuf.rearrange("p (t b) -> p t b", b=B)

            def load_chunk(k):
                stage = stagep.tile([128, CHUNK * B], fp32, tag="stage")
                nc.sync.dma_start(
                    out=stage, in_=et_d[:, k * CHUNK:(k + 1) * CHUNK, :]
                )
                nc.scalar.activation(
                    ebuf3[:, k * CHUNK:(k + 1) * CHUNK, :], stage, AF.Exp,
                    bias=negchat_bc,
                )

            # chunk issue order: chain A eats t_local 0..143 (chunks 0..17),
            # chain B eats t_local 128..271 (chunks 16..33)
            chunk_order = []
            for k in range(16):
                chunk_order.append(k)
                if 16 + k < NCHUNK:
                    chunk_order.append(16 + k)
            chunk_order += [32, 33]
            seen = set()
            chunk_order = [k for k in chunk_order
                           if not (k in seen or seen.add(k))]

            # ---------------- gold side work (fp8 DoubleRow) ----------------
            gsum = consts.tile([128, 2 * BC], fp32)
            gold_tiles = {}
            gold_cd = {}

            def gold_load(b):
                oh = ohstp.tile([128, NPAIR, 2, T], fp8, tag="oh")
                nc.sync.dma_start(out=oh, in_=oh_d[b])
                pr = prstp.tile([128, NPAIR, 2, 2 * T], fp8, tag="pr")
                nc.sync.dma_start(out=pr, in_=pr_d[b])
                gold_tiles[b] = (oh, pr)

            def gold_mm(b, p):
                if p == 0:
                    gold_cd[b] = cd_ps.tile(
                        [128, 256], fp32, tag="cd", name=f"cd{b}"
                    )
                oh, pr = gold_tiles[b]
                nc.tensor.matmul(
                    gold_cd[b], oh[:, p, :, :], pr[:, p, :, :],
                    start=(p == 0), stop=(p == NPAIR - 1), perf_mode=DR,
                )

            def gold_fin(b):
                cdump = dumpp.tile([128, 256], fp32, tag="cdump")
                nc.vector.tensor_tensor(
                    out=cdump, in0=gold_cd[b], in1=tri, op=ALU.mult
                )
                nc.vector.tensor_reduce(
                    gsum[:, 2 * b:2 * b + 2],
                    cdump.rearrange("p (c j) -> p c j", c=2),
                    axis=AX.X, op=ALU.add,
                )

            # strip DMAs run one batch row ahead of their matmuls
            side = [("L", 0, 0), ("L", 1, 0)]
            for b in range(BC):
                for p in range(NPAIR):
                    side.append(("M", b, p))
                side.append(("F", b, 0))
                if b + 2 < BC:
                    side.append(("L", b + 2, 0))

            def do_side(n):
                for _ in range(n):
                    if side:
                        kind, b, p = side.pop(0)
                        if kind == "L":
                            gold_load(b)
                        elif kind == "M":
                            gold_mm(b, p)
                        else:
                            gold_fin(b)

            # ---------------- chains ----------------
            pre = 6
            for k in chunk_order[:pre]:
                load_chunk(k)
            next_chunk = pre
            do_side(2)  # first two gold strip DMAs in flight early

            sA = statep.tile([128, B], bf16, tag="sA", name="sA0")
            nc.vector.memset(sA, 1.0)
            sB = statep.tile([128, B], bf16, tag="sB", name="sB0")
            nc.vector.memset(sB, 1.0)
            # parked colsums: [A_start | A_end | B_start | B_end]
            parks = consts.tile([1, 4 * B], fp32)

            def park(idx, st):
                cs = m_ps.tile([1, B], fp32, tag="m")
                nc.tensor.matmul(cs, ones_col_bf, st, start=True, stop=True)
                nc.vector.tensor_copy(parks[:, idx * B:(idx + 1) * B], cs)

            for r in range(NROT):
                qA = q_ps.tile([128, B], fp32, tag="q")
                nc.tensor.matmul(qA, w_bf, sA, start=True, stop=True)
                qB = q_ps.tile([128, B], fp32, tag="q")
                nc.tensor.matmul(qB, w_bf, sB, start=True, stop=True)
                nsA = statep.tile([128, B], bf16, tag="sA")
                nc.vector.tensor_tensor(
                    out=nsA, in0=qA, in1=ebuf3[:, r, :], op=ALU.mult
                )
                nsB = statep.tile([128, B], bf16, tag="sB")
                nc.vector.tensor_tensor(
                    out=nsB, in0=qB, in1=ebuf3[:, LSEG + r, :], op=ALU.mult
                )
                sA, sB = nsA, nsB
                if r == WU - 1:
                    park(0, sA)
                    park(2, sB)
                if r == NROT - 1:
                    park(1, sA)
                    park(3, sB)
                if r % 4 == 1 and next_chunk < NCHUNK:
                    load_chunk(chunk_order[next_chunk])
                    next_chunk += 1
                do_side(1)

            while next_chunk < NCHUNK:
                load_chunk(chunk_order[next_chunk])
                next_chunk += 1
            do_side(len(side))

            # ---------------- epilogue ----------------
            lnparks = smallp.tile([1, 4 * B], fp32, tag="lnp")
            nc.scalar.activation(lnparks, parks, AF.Ln)
            part = smallp.tile([1, B], fp32, tag="part")
            nc.vector.tensor_tensor(
                out=part, in0=lnparks[:, B:2 * B], in1=lnparks[:, 0:B],
                op=ALU.subtract,
            )
            nc.vector.tensor_tensor(
                out=part, in0=part, in1=lnparks[:, 3 * B:4 * B], op=ALU.add
            )
            nc.vector.tensor_tensor(
                out=part, in0=part, in1=lnparks[:, 2 * B:3 * B], op=ALU.subtract
            )
            nc.vector.tensor_scalar(
                out=part, in0=part, scalar1=chat256, scalar2=None, op0=ALU.add
            )

            # gold seq per local b: gsum cols [2b] = sum(C*trans), [2b+1] = esel
            gs_ps = m_ps.tile([1, 2 * BC], fp32, tag="m")
            nc.tensor.matmul(gs_ps, ones_col_f, gsum, start=True, stop=True)
            gs_sb = smallp.tile([1, 2 * BC], fp32, tag="gs")
            nc.vector.tensor_copy(gs_sb, gs_ps)
            seq2 = gs_sb.rearrange("p (b c) -> p b c", c=2)
            seq = smallp.tile([1, BC], fp32, tag="seq")
            nc.vector.tensor_tensor(
                out=seq, in0=seq2[:, :, 0], in1=seq2[:, :, 1], op=ALU.add
            )

            res = smallp.tile([1, B + BC], fp32, tag="res")
            nc.vector.tensor_copy(res[:, 0:B], part)
            nc.vector.tensor_copy(res[:, B:B + BC], seq)
            nc.sync.dma_start(out=out_d[:], in_=res[0:1, :])

    return nc


def _get_compiled(finalized=False):
    global _compiled
    if _compiled is None:
        _compiled = _build_program()
    if finalized and not _compiled.is_finalized():
        _compiled.finalize()
    return _compiled


def make_in_maps(emissions, transitions, tags):
    import ml_dtypes
    fp8 = ml_dtypes.float8_e4m3

    emissions = np.ascontiguousarray(emissions, dtype=np.float32)
    tags = np.asarray(tags).astype(np.int32)
    # transposed layout [T, S, B] for the chain's E slices
    et = np.ascontiguousarray(emissions.transpose(2, 1, 0))
    # shifted tags; 255 one-hot-encodes to all-zeros (no successor at s=S-1)
    tagsh = np.concatenate(
        [tags[:, 1:], np.full((B, 1), 255, dtype=np.int32)], axis=1
    )
    rng128 = np.arange(T, dtype=np.int32)
    emis8 = emissions.astype(fp8)

    in_maps = []
    for c in range(NCORES):
        lo = c * 2 * LSEG - WU
        hi = c * 2 * LSEG + 2 * LSEG
        if lo < 0:
            pad = np.repeat(et[:, 0:1, :], -lo, axis=1)
            sl = np.concatenate([pad, et[:, 0:hi, :]], axis=1)
        else:
            sl = et[:, lo:hi, :]
        bsl = slice(c * BC, (c + 1) * BC)
        # gold strips in DoubleRow layout [b, s, pair, ktile, cols]
        tg = tags[bsl].reshape(BC, NPAIR, 2, 128)        # [b, pair, kt, s]
        oh = (tg[..., None] == rng128).astype(fp8)       # [b, pair, kt, s, tag]
        oh_pack = np.ascontiguousarray(oh.transpose(0, 3, 1, 2, 4))
        tsh = tagsh[bsl].reshape(BC, NPAIR, 2, 128)
        ohs = (tsh[..., None] == rng128).astype(fp8)
        em8 = emis8[bsl].reshape(BC, NPAIR, 2, 128, T)
        pair = np.concatenate([ohs, em8], axis=4)        # [b, pair, kt, s, 2T]
        pair_pack = np.ascontiguousarray(pair.transpose(0, 3, 1, 2, 4))
        in_maps.append({
            "emis_t": np.ascontiguousarray(sl),
            "transitions": np.ascontiguousarray(transitions, dtype=np.float32),
            "oh_pack": oh_pack,
            "pair_pack": pair_pack,
        })
    return in_maps


def _run_device(emissions, transitions, tags):
    from concourse.bass_utils import run_bass_kernel_spmd

    nc = _get_compiled(finalized=True)
    res = run_bass_kernel_spmd(
        nc, make_in_maps(emissions, transitions, tags), list(range(NCORES))
    )
    outs = [res.results[c]["loss_parts"] for c in range(NCORES)]
    logZ = np.sum([o[:B] for o in outs], axis=0)
    seq = np.concatenate([o[B:] for o in outs])
    return np.float32((logZ - seq).mean())


def _run_host(emissions, transitions, tags, mask):
    """Slow but fully general fallback (any mask pattern)."""
    e = emissions.astype(np.float64)
    t = transitions.astype(np.float64)

    def lse(x, axis):
        m = x.max(axis=axis, keepdims=True)
        return (m + np.log(np.exp(x - m).sum(axis=axis, keepdims=True))).squeeze(axis)

    score = e[:, 0]
    for s in range(1, e.shape[1]):
        nxt = lse(score[:, :, None] + t[None, :, :] + e[:, s, None, :], axis=1)
        score = np.where(mask[:, s, None], nxt, score)
    log_Z = lse(score, axis=1)
    emit = np.take_along_axis(e, tags[..., None].astype(np.int64), axis=2)[..., 0]
    trans_sc = t[tags[:, :-1].astype(np.int64), tags[:, 1:].astype(np.int64)]
    m = mask[:, 1:].astype(np.float64)
    seq = emit[:, 0] + ((trans_sc + emit[:, 1:]) * m).sum(axis=1)
    return np.float32((log_Z - seq).mean())


def kernel(emissions, transitions, tags, mask):
    emissions = np.asarray(emissions)
    transitions = np.asarray(transitions)
    tags = np.asarray(tags)
    mask = np.asarray(mask)
    if emissions.shape != (B, S, T) or not mask.all():
        return _run_host(emissions, transitions, tags, mask)
    return _run_device(emissions, transitions, tags)


# revision 9
# speedup vs baseline: 5.8447x; 1.1957x over previous
"""Trainium2 Bass kernel for a batched linear-chain CRF negative log-likelihood.

reference semantics (B=128, S=2048, T=128):
    forward algorithm over S steps (log-space matvec chain) -> log_Z per batch
    gold path score = emissions gathered at tags + transitions gathered at
    (tag_t, tag_{t+1}) pairs, summed over time
    output = mean(log_Z - seq_score)   (scalar f32)

Strategy (v4 — sequence-parallel chain, 2x256-wide streams, fp8 gold):
  - The linear-space forward recursion a_t = (a_{t-1} @ W) * E_t is a product
    of strictly positive matrices, which contracts to rank-1 at ~10x per step
    (Birkhoff).  A chain warm-started from a uniform vector ~12 steps before a
    segment boundary carries the true state *direction* to below bf16 noise,
    and log Z telescopes into per-segment colsum differences:
        log Z = sum_k [ln colsum(a at seg_k end) - ln colsum(a at seg_k start)]
    evaluated on each segment's own warm-started trajectory.
  - S=2048 is split into 32 segments of 64 steps; each of the 8 cores runs its
    4 segments as 2 interleaved STREAMS, each stream carrying 2 segments
    side-by-side in a [tag=128, 2*batch=256] state: per rotation one bf16
    matmul (stationary W = exp(transitions) shared by everything) and one DVE
    multiply per stream.  76 rotations per core; the two streams hide each
    other's PE->DVE->PE round-trip latency.
  - No renormalization: E_t = exp(emit_t - chat2) with chat2 = mean ln colsum W
    + 0.5 (the +0.5 cancels the lognormal emission mean-growth); state log
    magnitude stays within ~[-9, +16] over a 76-step unrenormalized chain.
  - E is produced with zero PE work: the host gathers the transposed
    emissions into the exact [tag, rotation, chain, batch] consumption order,
    so the device DMAs contiguous fp32 chunks and runs one wide scalar-engine
    exp per chunk straight into the bf16 E buffer.
  - Gold path batch-sharded, fp8: the host re-encodes tags as fp8 one-hot
    strips (pure index marshalling) packed in the DoubleRow two-k-tile
    layout, so each batch row needs 2 DMAs and 8 fp8 DoubleRow matmuls
    (256-deep contraction each):  CD_b += OH^T @ [OHshift | EMIS], then one
    DVE multiply by [trans | identity] and a grouped reduce.  fp8 is exact
    for the 0/1 one-hots and the count matrix; fp8 rounding of emissions
    perturbs the loss by ~4e-7 relative (tol 2e-2).
  - Per-core output: per-batch chain partials (sum of its 4 segments,
    + 256*chat2) and the 16 gold sequence scores for its batch shard; host
    sums partials across cores and takes the mean.
"""

import numpy as np

B, S, T = 128, 2048, 128
NCORES = 8
BC = B // NCORES          # 16 batch rows per core (gold shard)
NSB = S // 128            # 16 s-blocks of 128
NPAIR = NSB // 2          # 8 DoubleRow block-pairs
NCH = 4                   # chains per core
LSEG = S // (NCORES * NCH)  # 64-step segments
WU = 12                   # warm-up steps per chain
NROT = LSEG + WU          # 76 rotations
CROT = 2                  # rotations per E chunk
NCHUNK = NROT // CROT     # 38

_compiled = None


def _build_program():
    import concourse.bass as bass
    import concourse.bacc as bacc
    import concourse.tile as tile
    from concourse import mybir
    from concourse.masks import make_identity

    fp32 = mybir.dt.float32
    bf16 = mybir.dt.bfloat16
    fp8 = mybir.dt.float8e4
    AF = mybir.ActivationFunctionType
    ALU = mybir.AluOpType
    AX = mybir.AxisListType
    DR = mybir.MatmulPerfMode.DoubleRow

    nc = bacc.Bacc(None)
    # E inputs pre-gathered on host into consumption order [tag, rot, chain, b]
    et_d = nc.declare_dram_parameter("emis_t", [T, NROT, NCH, B], fp32,
                                     isOutput=False)
    tr_d = nc.declare_dram_parameter("transitions", [T, T], fp32, isOutput=False)
    # gold fp8 strips, DoubleRow layout: [b, s, pair, ktile, cols]
    oh_d = nc.declare_dram_parameter("oh_pack", [BC, 128, NPAIR, 2, T], fp8,
                                     isOutput=False)
    pr_d = nc.declare_dram_parameter("pair_pack", [BC, 128, NPAIR, 2, 2 * T],
                                     fp8, isOutput=False)
    out_d = nc.declare_dram_parameter("loss_parts", [B + BC], fp32, isOutput=True)

    with tile.TileContext(nc) as tc:
        with (
            tc.tile_pool(name="consts", bufs=1) as consts,
            tc.tile_pool(name="ebuf", bufs=1) as ebufp,
            tc.tile_pool(name="stage", bufs=3) as stagep,
            tc.tile_pool(name="ohst", bufs=3) as ohstp,
            tc.tile_pool(name="prst", bufs=3) as prstp,
            tc.tile_pool(name="dump", bufs=4) as dumpp,
            tc.tile_pool(name="state", bufs=6) as statep,
            tc.tile_pool(name="small", bufs=8) as smallp,
            tc.tile_pool(name="q_ps", bufs=4, space="PSUM") as q_ps,
            tc.tile_pool(name="cd_ps", bufs=2, space="PSUM") as cd_ps,
            tc.tile_pool(name="m_ps", bufs=1, space="PSUM") as m_ps,
        ):
            # ---------------- constants ----------------
            ident = consts.tile([128, 128], fp32)
            make_identity(nc, ident)
            ones_col_bf = consts.tile([128, 1], bf16)
            nc.vector.memset(ones_col_bf, 1.0)
            ones_col_f = consts.tile([128, 1], fp32)
            nc.vector.memset(ones_col_f, 1.0)
            ones_row_f = consts.tile([1, 128], fp32)
            nc.vector.memset(ones_row_f, 1.0)

            # transitions -> W = exp(trans) bf16 (chain stationary)
            tr_sb = consts.tile([128, 128], fp32)
            nc.sync.dma_start(out=tr_sb, in_=tr_d[:, :])
            w_bf = consts.tile([128, 128], bf16)
            nc.scalar.activation(w_bf, tr_sb, AF.Exp)

            # [trans | identity] for the gold finalize
            tri = consts.tile([128, 256], fp32)
            nc.vector.tensor_copy(tri[:, 0:128], tr_sb)
            nc.vector.tensor_copy(tri[:, 128:256], ident)

            # chat2 = mean_j ln(colsum_j W) over j=1..127, + 0.5
            colw_ps = m_ps.tile([1, 128], fp32, tag="m")
            nc.tensor.matmul(colw_ps, ones_col_bf, w_bf, start=True, stop=True)
            lncol = smallp.tile([1, 127], fp32, tag="lncol")
            lnsum = consts.tile([1, 1], fp32)
            nc.scalar.activation(lncol, colw_ps[:, 1:128], AF.Ln, accum_out=lnsum)
            negchat = smallp.tile([1, 1], fp32, tag="nch")
            nc.scalar.activation(negchat, lnsum, AF.Copy, scale=-1.0 / 127.0)
            nc.vector.tensor_scalar(
                out=negchat, in0=negchat, scalar1=-0.5, scalar2=None, op0=ALU.add
            )
            nbc_ps = m_ps.tile([128, 1], fp32, tag="m")
            nc.tensor.matmul(nbc_ps, ones_row_f, negchat, start=True, stop=True)
            negchat_bc = consts.tile([128, 1], fp32)
            nc.vector.tensor_copy(negchat_bc, nbc_ps)
            # 256*chat2 = lnsum*(256/127) + 128
            chat256 = consts.tile([1, 1], fp32)
            nc.scalar.activation(chat256, lnsum, AF.Copy, scale=256.0 / 127.0)
            nc.vector.tensor_scalar(
                out=chat256, in0=chat256, scalar1=128.0, scalar2=None, op0=ALU.add
            )

            # ---------------- E buffer + loading ----------------
            ebuf = ebufp.tile([128, NROT * NCH * B], bf16)
            ebuf4 = ebuf.rearrange("p (r j b) -> p r j b", j=NCH, b=B)

            def load_chunk(k):
                stage = stagep.tile([128, CROT * NCH * B], fp32, tag="stage")
                nc.sync.dma_start(
                    out=stage, in_=et_d[:, k * CROT:(k + 1) * CROT, :, :]
                )
                nc.scalar.activation(
                    ebuf4[:, k * CROT:(k + 1) * CROT, :, :], stage, AF.Exp,
                    bias=negchat_bc,
                )

            # ---------------- gold side work (fp8 DoubleRow) ----------------
            gsum = consts.tile([128, 2 * BC], fp32)
            gold_tiles = {}
            gold_cd = {}

            def gold_load(b):
                oh = ohstp.tile([128, NPAIR, 2, T], fp8, tag="oh")
                nc.sync.dma_start(out=oh, in_=oh_d[b])
                pr = prstp.tile([128, NPAIR, 2, 2 * T], fp8, tag="pr")
                nc.sync.dma_start(out=pr, in_=pr_d[b])
                gold_tiles[b] = (oh, pr)

            def gold_mm(b, p):
                if p == 0:
                    gold_cd[b] = cd_ps.tile(
                        [128, 256], fp32, tag="cd", name=f"cd{b}"
                    )
                oh, pr = gold_tiles[b]
                nc.tensor.matmul(
                    gold_cd[b], oh[:, p, :, :], pr[:, p, :, :],
                    start=(p == 0), stop=(p == NPAIR - 1), perf_mode=DR,
                )

            def gold_fin(b):
                cdump = dumpp.tile([128, 256], fp32, tag="cdump")
                nc.vector.tensor_tensor(
                    out=cdump, in0=gold_cd[b], in1=tri, op=ALU.mult
                )
                nc.vector.tensor_reduce(
                    gsum[:, 2 * b:2 * b + 2],
                    cdump.rearrange("p (c j) -> p c j", c=2),
                    axis=AX.X, op=ALU.add,
                )

            # strip DMAs run one batch row ahead of their matmuls
            side = [("L", 0, 0), ("L", 1, 0)]
            for b in range(BC):
                for p in range(NPAIR):
                    side.append(("M", b, p))
                side.append(("F", b, 0))
                if b + 2 < BC:
                    side.append(("L", b + 2, 0))

            def do_side(n):
                for _ in range(n):
                    if side:
                        kind, b, p = side.pop(0)
                        if kind == "L":
                            gold_load(b)
                        elif kind == "M":
                            gold_mm(b, p)
                        else:
                            gold_fin(b)

            # ---------------- chains: 2 streams of [128, 256] ----------------
            pre = 6
            for k in range(pre):
                load_chunk(k)
            next_chunk = pre
            do_side(2)  # first two gold strip DMAs in flight early

            st = []
            for j in range(2):
                s0 = statep.tile([128, 2 * B], bf16, tag=f"s{j}", name=f"s{j}_0")
                nc.vector.memset(s0, 1.0)
                st.append(s0)
            # parked colsums: [s0 start | s0 end | s1 start | s1 end]
            parks = consts.tile([1, 8 * B], fp32)

            def park(idx, s):
                cs = m_ps.tile([1, 2 * B], fp32, tag="m")
                nc.tensor.matmul(cs, ones_col_bf, s, start=True, stop=True)
                nc.vector.tensor_copy(parks[:, idx * 2 * B:(idx + 1) * 2 * B], cs)

            for r in range(NROT):
                q = []
                for j in range(2):
                    qj = q_ps.tile([128, 2 * B], fp32, tag="q")
                    nc.tensor.matmul(qj, w_bf, st[j], start=True, stop=True)
                    q.append(qj)
                for j in range(2):
                    ns = statep.tile([128, 2 * B], bf16, tag=f"s{j}")
                    nc.vector.tensor_tensor(
                        out=ns, in0=q[j], in1=ebuf4[:, r, 2 * j:2 * j + 2, :],
                        op=ALU.mult,
                    )
                    st[j] = ns
                if r == WU - 1:
                    park(0, st[0])
                    park(2, st[1])
                if r == NROT - 1:
                    park(1, st[0])
                    park(3, st[1])
                if r % 2 == 1 and next_chunk < NCHUNK:
                    load_chunk(next_chunk)
                    next_chunk += 1
                do_side(2)

            while next_chunk < NCHUNK:
                load_chunk(next_chunk)
                next_chunk += 1
            do_side(len(side))

            # ---------------- epilogue ----------------
            lnparks = smallp.tile([1, 8 * B], fp32, tag="lnp")
            nc.scalar.activation(lnparks, parks, AF.Ln)
            d0 = smallp.tile([1, 2 * B], fp32, tag="d0")
            nc.vector.tensor_tensor(
                out=d0, in0=lnparks[:, 2 * B:4 * B], in1=lnparks[:, 0:2 * B],
                op=ALU.subtract,
            )
            d1 = smallp.tile([1, 2 * B], fp32, tag="d1")
            nc.vector.tensor_tensor(
                out=d1, in0=lnparks[:, 6 * B:8 * B], in1=lnparks[:, 4 * B:6 * B],
                op=ALU.subtract,
            )
            part = smallp.tile([1, B], fp32, tag="part")
            nc.vector.tensor_tensor(
                out=part, in0=d0[:, 0:B], in1=d0[:, B:2 * B], op=ALU.add
            )
            nc.vector.tensor_tensor(
                out=part, in0=part, in1=d1[:, 0:B], op=ALU.add
            )
            nc.vector.tensor_tensor(
                out=part, in0=part, in1=d1[:, B:2 * B], op=ALU.add
            )
            nc.vector.tensor_scalar(
                out=part, in0=part, scalar1=chat256, scalar2=None, op0=ALU.add
            )

            # gold seq per local b: gsum cols [2b] = sum(C*trans), [2b+1] = esel
            gs_ps = m_ps.tile([1, 2 * BC], fp32, tag="m")
            nc.tensor.matmul(gs_ps, ones_col_f, gsum, start=True, stop=True)
            gs_sb = smallp.tile([1, 2 * BC], fp32, tag="gs")
            nc.vector.tensor_copy(gs_sb, gs_ps)
            seq2 = gs_sb.rearrange("p (b c) -> p b c", c=2)
            seq = smallp.tile([1, BC], fp32, tag="seq")
            nc.vector.tensor_tensor(
                out=seq, in0=seq2[:, :, 0], in1=seq2[:, :, 1], op=ALU.add
            )

            res = smallp.tile([1, B + BC], fp32, tag="res")
            nc.vector.tensor_copy(res[:, 0:B], part)
            nc.vector.tensor_copy(res[:, B:B + BC], seq)
            nc.sync.dma_start(out=out_d[:], in_=res[0:1, :])

    return nc


def _get_compiled(finalized=False):
    global _compiled
    if _compiled is None:
        _compiled = _build_program()
    if finalized and not _compiled.is_finalized():
        _compiled.finalize()
    return _compiled


def make_in_maps(emissions, transitions, tags):
    import ml_dtypes
    fp8 = ml_dtypes.float8_e4m3

    emissions = np.ascontiguousarray(emissions, dtype=np.float32)
    tags = np.asarray(tags).astype(np.int32)
    # transposed layout [T, S, B], then gathered into consumption order
    et = np.ascontiguousarray(emissions.transpose(2, 1, 0))
    # shifted tags; 255 one-hot-encodes to all-zeros (no successor at s=S-1)
    tagsh = np.concatenate(
        [tags[:, 1:], np.full((B, 1), 255, dtype=np.int32)], axis=1
    )
    rng128 = np.arange(T, dtype=np.int32)
    emis8 = emissions.astype(fp8)

    rr = np.arange(NROT)[:, None]                  # [rot, 1]
    jj = np.arange(NCH)[None, :]                   # [1, chain]
    in_maps = []
    for c in range(NCORES):
        # E gather: slice index per (rotation, chain); clip<0 repeats slice 0
        idx = np.clip(c * NCH * LSEG + jj * LSEG - WU + rr, 0, S - 1)
        sl = np.ascontiguousarray(et[:, idx, :])   # [T, NROT, NCH, B]
        bsl = slice(c * BC, (c + 1) * BC)
        # gold strips in DoubleRow layout [b, s, pair, ktile, cols]
        tg = tags[bsl].reshape(BC, NPAIR, 2, 128)
        oh = (tg[..., None] == rng128).astype(fp8)
        oh_pack = np.ascontiguousarray(oh.transpose(0, 3, 1, 2, 4))
        tsh = tagsh[bsl].reshape(BC, NPAIR, 2, 128)
        ohs = (tsh[..., None] == rng128).astype(fp8)
        em8 = emis8[bsl].reshape(BC, NPAIR, 2, 128, T)
        pair = np.concatenate([ohs, em8], axis=4)
        pair_pack = np.ascontiguousarray(pair.transpose(0, 3, 1, 2, 4))
        in_maps.append({
            "emis_t": sl,
            "transitions": np.ascontiguousarray(transitions, dtype=np.float32),
            "oh_pack": oh_pack,
            "pair_pack": pair_pack,
        })
    return in_maps


def _run_device(emissions, transitions, tags):
    from concourse.bass_utils import run_bass_kernel_spmd

    nc = _get_compiled(finalized=True)
    res = run_bass_kernel_spmd(
        nc, make_in_maps(emissions, transitions, tags), list(range(NCORES))
    )
    outs = [res.results[c]["loss_parts"] for c in range(NCORES)]
    logZ = np.sum([o[:B] for o in outs], axis=0)
    seq = np.concatenate([o[B:] for o in outs])
    return np.float32((logZ - seq).mean())


def _run_host(emissions, transitions, tags, mask):
    """Slow but fully general fallback (any mask pattern)."""
    e = emissions.astype(np.float64)
    t = transitions.astype(np.float64)

    def lse(x, axis):
        m = x.max(axis=axis, keepdims=True)
        return (m + np.log(np.exp(x - m).sum(axis=axis, keepdims=True))).squeeze(axis)

    score = e[:, 0]
    for s in range(1, e.shape[1]):
        nxt = lse(score[:, :, None] + t[None, :, :] + e[:, s, None, :], axis=1)
        score = np.where(mask[:, s, None], nxt, score)
    log_Z = lse(score, axis=1)
    emit = np.take_along_axis(e, tags[..., None].astype(np.int64), axis=2)[..., 0]
    trans_sc = t[tags[:, :-1].astype(np.int64), tags[:, 1:].astype(np.int64)]
    m = mask[:, 1:].astype(np.float64)
    seq = emit[:, 0] + ((trans_sc + emit[:, 1:]) * m).sum(axis=1)
    return np.float32((log_Z - seq).mean())


def kernel(emissions, transitions, tags, mask):
    emissions = np.asarray(emissions)
    transitions = np.asarray(transitions)
    tags = np.asarray(tags)
    mask = np.asarray(mask)
    if emissions.shape != (B, S, T) or not mask.all():
        return _run_host(emissions, transitions, tags, mask)
    return _run_device(emissions, transitions, tags)
